# revision 5
# baseline (speedup 1.0000x reference)
"""Trainium2 Bass kernel: 31x31 SAME conv of 12 (=4*3) 64x64 maps with 128 filters.

out[m, co, y, x] = sum_{ky,kx} F[co,ky,kx] * Ipad[m, y+ky, x+kx]   (pad 15)

Scheme (per core; 8 cores split the 64 output rows, 8 rows each):
  - SBUF X tile [128, 3, 39, 95]: partition p = 32*g + j holds, for each of
    image-group g's 3 images, rows y0..y0+38 of the padded image shifted
    left by j columns (j = kx tap; j=31 is a zero-weight pad partition).
  - SBUF W tile [128, 31, 128]: partition 32*g + j holds F[co, ky, j] for
    all (ky, co); replicated across the 4 groups g.
  - For each local image i (3) and ky (31): 4 concurrent row-tiled matmuls
    (tile_position=(32g,0), K=32 = kx taps) accumulate into 4 PSUM banks
    (one per group = one per image), N = 512 = 8 rows x 64 cols.
  - PSUM -> SBUF copy (VectorE), DMA out.

Matmul dtype is float32r (full fp32 bits, 1 cycle/row at N>=512).
"""

import os
import numpy as np

# ---- problem constants (hardcoded; must match the grader's reference) ----
B, F, H, W = 4, 3, 64, 64
M = B * F                      # 12 independent maps
NCO = 128                      # output channels (8 angles * 16 taus)
FW = 31                        # filter width
PAD = 15
NCORES = 8
RPC = H // NCORES              # 8 output rows per core
NG = 4                         # partition groups (row-tiles)
IPG = M // NG                  # images per group = 3
SROWS = RPC + FW               # 39 stored padded rows per (partition, image)
SCOLS = 95                     # stored cols per row (64 + 31)
NTILE = RPC * W                # 512 = PSUM tile free size

MM_DTYPE = os.environ.get("BASS_MM_DTYPE", "float32r")

_CACHE = {}


def _build_program(mm_dtype=MM_DTYPE):
    key = mm_dtype
    if key in _CACHE:
        return _CACHE[key]
    from concourse import bacc, tile, mybir

    mmdt = getattr(mybir.dt, mm_dtype)
    nc = bacc.Bacc("TRN2", target_bir_lowering=False, debug=False,
                   num_devices=NCORES, enable_asserts=False)
    xin = nc.dram_tensor("xin", [128, IPG, SROWS, SCOLS], mmdt,
                         kind="ExternalInput").ap()
    win = nc.dram_tensor("win", [128, FW, NCO], mmdt,
                         kind="ExternalInput").ap()
    out = nc.dram_tensor("out", [NG, IPG, NCO, NTILE], mybir.dt.float32,
                         kind="ExternalOutput").ap()

    with tile.TileContext(nc) as tc:
        with tc.tile_pool(name="wpool", bufs=1) as wp, \
             tc.tile_pool(name="xpool", bufs=2) as xp, \
             tc.tile_pool(name="opool", bufs=4) as op, \
             tc.tile_pool(name="psum", bufs=8, space="PSUM") as pp:
            wt = wp.tile([128, FW, NCO], mmdt)
            nc.sync.dma_start(out=wt, in_=win)
            for i in range(IPG):
                xt = xp.tile([128, SROWS, SCOLS], mmdt)
                nc.sync.dma_start(out=xt, in_=xin[:, i])
                psums = [pp.tile([NCO, NTILE], mybir.dt.float32,
                                 name=f"ps_{i}_{g}", tag="ps")
                         for g in range(NG)]
                for ky in range(FW):
                    for g in range(NG):
                        lhsT = wt[32 * g:32 * g + 32, ky, :]
                        rhs = xt[32 * g:32 * g + 32, ky:ky + RPC, 0:W]
                        nc.tensor.matmul(
                            psums[g][:, :],
                            lhsT,
                            rhs,
                            start=(ky == 0),
                            stop=(ky == FW - 1),
                            tile_position=(32 * g, 0),
                        )
                for g in range(NG):
                    ot = op.tile([NCO, NTILE], mybir.dt.float32)
                    nc.vector.tensor_copy(out=ot, in_=psums[g])
                    nc.sync.dma_start(out=out[g, i], in_=ot)
    nc.compile()
    _CACHE[key] = nc
    return nc


def _host_prep(inp, filters):
    """Build per-core xin arrays and the shared win array."""
    imgs = np.ascontiguousarray(inp.reshape(M, H, W), dtype=np.float32)
    # rows: need y0..y0+38 with y0 max 56 -> 95 rows; cols: j+c <= 125 -> 126
    ipad = np.zeros((M, 95, 126), dtype=np.float32)
    ipad[:, PAD:PAD + H, PAD:PAD + W] = imgs

    xins = []
    for c in range(NCORES):
        y0 = RPC * c
        x4 = np.empty((128, IPG, SROWS, SCOLS), dtype=np.float32)
        for g in range(NG):
            sl = ipad[IPG * g:IPG * (g + 1), y0:y0 + SROWS, :]  # (3, 39, 126)
            for j in range(32):
                x4[32 * g + j] = sl[:, :, j:j + SCOLS]
        xins.append(x4)

    w = np.zeros((128, FW, NCO), dtype=np.float32)
    ftap = np.ascontiguousarray(
        filters.reshape(NCO, FW, FW).transpose(2, 1, 0), dtype=np.float32
    )  # [j, ky, co]
    for g in range(NG):
        w[32 * g:32 * g + FW] = ftap
    return xins, w


def _assemble(results):
    """results: list of 8 dicts with 'out' [NG, IPG, NCO, NTILE]."""
    conv = np.empty((M, NCO, H, W), dtype=np.float32)
    for c in range(NCORES):
        oc = results[c]["out"].reshape(NG, IPG, NCO, RPC, W)
        for g in range(NG):
            for i in range(IPG):
                conv[IPG * g + i, :, RPC * c:RPC * (c + 1), :] = oc[g, i]
    out = conv.reshape(B, F, 8, 16, H, W).transpose(0, 1, 3, 2, 4, 5)
    return np.ascontiguousarray(out)


LAST_RESULT = None


def kernel(inp, filters):
    global LAST_RESULT
    inp = np.asarray(inp, dtype=np.float32)
    filters = np.asarray(filters, dtype=np.float32)
    xins, w = _host_prep(inp, filters)
    nc = _build_program()
    if MM_DTYPE == "bfloat16":
        import ml_dtypes
        xins = [x.astype(ml_dtypes.bfloat16) for x in xins]
        w = w.astype(ml_dtypes.bfloat16)
    in_maps = [{"xin": xins[c], "win": w} for c in range(NCORES)]

    if os.environ.get("BASS_KERNEL_SIM"):
        from concourse.bass_interp import CoreSim
        results = []
        for c in range(NCORES):
            sim = CoreSim(nc)
            for k, v in in_maps[c].items():
                sim.tensor(k)[:] = v
            sim.simulate()
            results.append({"out": sim.tensor("out").copy()})
    else:
        from concourse.bass_utils import run_bass_kernel_spmd
        trace = bool(os.environ.get("BASS_KERNEL_TRACE"))
        res = run_bass_kernel_spmd(nc, in_maps, list(range(NCORES)),
                                   trace=trace)
        LAST_RESULT = res
        results = res.results
    return _assemble(results)


# revision 23
# speedup vs baseline: 4.1810x; 4.1810x over previous
"""Trainium2 Bass kernel: 31x31 SAME conv of 12 (=4*3) 64x64 maps with 128 filters.

out[m, co, y, x] = sum_{ky,kx} F[co,ky,kx] * Ipad[m, y+ky, x+kx]   (pad 15)

Scheme (per core; 8 cores split the 64 output rows, 8 rows each, y0 = 8*core):
  - The 961 filter taps are contracted in 8 chunks of K=124 = (ky in 0..30) x
    (dx in 0..3); chunk c covers kx = 4c + dx (kx=31 taps are zero weights).
  - SBUF X tile [124, 12, 8, 95]: partition p = ky*4 + dx holds, for each
    image m, the 8 rows y0+ky .. y0+ky+7 of the padded image, shifted left by
    dx columns.  The rhs view for (m, chunk c) is the pure slice
    X[0:124, m, 0:8, 4c:4c+64]  (free dims: 8 rows x 64 cols = 512 = N) --
    the kx chunk offset rides the free x dimension, so the 8 chunks share
    one stored copy.
  - SBUF W tile [124, 8, 128]: W[p, c, co] = F[co, ky, 4c+dx] (0 if kx>30).
  - Per image m: one PSUM tile [128co, 512] accumulates the 8 chunk matmuls;
    then VectorE copies PSUM->SBUF and DMA stores to out[m].
  - Host does the (ky, dx) shifted-window replication (pure numpy slicing)
    and the final gather/transpose to the reference layout.

Matmul dtype: float32r (fp32 bits at ~1 cycle/row for N>=256) by default;
BASS_MM_DTYPE=bfloat16|float32 selects variants.
"""

import os
import numpy as np

# ---- problem constants (hardcoded; must match the grader's reference) ----
B, F, H, W = 4, 3, 64, 64
M = B * F                      # 12 independent maps
NCO = 128                      # output channels (8 angles * 16 taus)
FW = 31                        # filter width
PAD = 15
NCORES = 8
RPC = H // NCORES              # 8 output rows per core
NDX = 4                        # kx sub-taps per chunk (col shifts)
NKP = FW * NDX                 # 124 contraction partitions
NCH = 8                        # chunks: ceil(31/4)
SCOLS = 95                     # stored cols per row (64 + 31)
NTILE = RPC * W                # 512 = PSUM tile free size

MM_DTYPE = os.environ.get("BASS_MM_DTYPE", "float32r")

_CACHE = {}


def _build_program(mm_dtype=MM_DTYPE):
    key = mm_dtype
    if key in _CACHE:
        return _CACHE[key]
    from concourse import bacc, tile, mybir

    mmdt = getattr(mybir.dt, mm_dtype)
    nc = bacc.Bacc("TRN2", target_bir_lowering=False, debug=False,
                   num_devices=NCORES, enable_asserts=False)
    xin = nc.dram_tensor("xin", [NKP, M, RPC, SCOLS], mmdt,
                         kind="ExternalInput").ap()
    win = nc.dram_tensor("win", [NKP, NCH, NCO], mmdt,
                         kind="ExternalInput").ap()
    out = nc.dram_tensor("out", [M, NCO, NTILE], mybir.dt.float32,
                         kind="ExternalOutput").ap()

    nwarm = int(os.environ.get("BASS_NWARM", "12"))
    with tile.TileContext(nc) as tc:
        with tc.tile_pool(name="wpool", bufs=1) as wp, \
             tc.tile_pool(name="xpool", bufs=1) as xp, \
             tc.tile_pool(name="opool", bufs=4) as op, \
             tc.tile_pool(name="wupool", bufs=1) as wup, \
             tc.tile_pool(name="psum", bufs=7, space="PSUM") as pp, \
             tc.tile_pool(name="pswarm", bufs=1, space="PSUM") as ppw:
            # PE warmup: a stream of tiny matmuls on a memset tile keeps the
            # PE busy (HAM warm / cost-model ramp) while the X/W DMAs land.
            if nwarm:
                wu = wup.tile([1, 64], mybir.dt.float32)
                nc.vector.memset(wu, 0.0)
                pw = ppw.tile([1, 64], mybir.dt.float32)
                for _ in range(nwarm):
                    nc.tensor.matmul(pw[:, :], wu[:, :1], wu,
                                     start=True, stop=True)
            # W as two tiles so the first matmuls only wait for half the
            # weights; DMA order w_a, x_0, w_b, x_1, ... pipelines the start.
            HCH = NCH // 2
            wta = wp.tile([NKP, HCH, NCO], mmdt)
            wtb = wp.tile([NKP, NCH - HCH, NCO], mmdt)
            nc.sync.dma_start(out=wta, in_=win[:, :HCH])
            xts = []
            for m in range(M):
                xt = xp.tile([NKP, RPC, SCOLS], mmdt,
                             name=f"xt_{m}", tag=f"xt{m}")
                nc.sync.dma_start(out=xt, in_=xin[:, m])
                xts.append(xt)
                if m == 0:
                    nc.sync.dma_start(out=wtb, in_=win[:, HCH:])

            def mm(ps, m, c, rows, cols, start, stop):
                wt = wta if c < HCH else wtb
                nc.tensor.matmul(
                    ps,
                    wt[:, c % HCH if c < HCH else c - HCH, :],
                    xts[m][:, rows, NDX * c:NDX * c + W][:, :, cols],
                    start=start, stop=stop,
                )

            for m in range(M - 1):
                ps = pp.tile([NCO, NTILE], mybir.dt.float32,
                             name=f"ps_{m}", tag="ps")
                for c in range(NCH):
                    mm(ps[:, :], m, c, slice(0, RPC), slice(0, W),
                       c == 0, c == NCH - 1)
                ot = op.tile([NCO, NTILE], mybir.dt.float32,
                             name=f"ot_{m}", tag="ot")
                nc.vector.tensor_copy(out=ot, in_=ps)
                nc.sync.dma_start(out=out[m], in_=ot)
            # last image in two half tiles to shorten the copy+store tail
            m = M - 1
            for h in range(2):
                rows = slice(h * RPC // 2, (h + 1) * RPC // 2)
                ps = pp.tile([NCO, NTILE // 2], mybir.dt.float32,
                             name=f"ps_{m}_{h}", tag="ps")
                for c in range(NCH):
                    mm(ps[:, :], m, c, rows, slice(0, W),
                       c == 0, c == NCH - 1)
                ot = op.tile([NCO, NTILE // 2], mybir.dt.float32,
                             name=f"ot_{m}_{h}", tag="oth")
                nc.vector.tensor_copy(out=ot, in_=ps)
                nc.sync.dma_start(
                    out=out[m][:, h * NTILE // 2:(h + 1) * NTILE // 2],
                    in_=ot)
    nc.compile()
    _CACHE[key] = nc
    return nc


def _host_prep(inp, filters):
    """Build per-core xin arrays and the shared win array."""
    imgs = np.ascontiguousarray(inp.reshape(M, H, W), dtype=np.float32)
    # rows: y0+ky+y <= 56+30+7 = 93 -> 94 rows; cols: dx+4c+x <= 97 -> 98
    ipad = np.zeros((M, 94, 98), dtype=np.float32)
    ipad[:, PAD:PAD + H, PAD:PAD + W] = imgs

    xins = []
    for core in range(NCORES):
        y0 = RPC * core
        x4 = np.empty((NKP, M, RPC, SCOLS), dtype=np.float32)
        for ky in range(FW):
            for dx in range(NDX):
                x4[ky * NDX + dx] = ipad[:, y0 + ky:y0 + ky + RPC,
                                         dx:dx + SCOLS]
        xins.append(x4)

    w = np.zeros((NKP, NCH, NCO), dtype=np.float32)
    fk = np.ascontiguousarray(filters.reshape(NCO, FW, FW), dtype=np.float32)
    for dx in range(NDX):
        for c in range(NCH):
            kx = NDX * c + dx
            if kx < FW:
                # w[ky*4+dx, c, co] = F[co, ky, kx]
                w[dx::NDX, c, :] = fk[:, :, kx].T
    return xins, w


def _assemble(results):
    """results: list of 8 dicts with 'out' [M, NCO, NTILE]."""
    conv = np.empty((M, NCO, H, W), dtype=np.float32)
    for c in range(NCORES):
        conv[:, :, RPC * c:RPC * (c + 1), :] = \
            results[c]["out"].reshape(M, NCO, RPC, W)
    out = conv.reshape(B, F, 8, 16, H, W).transpose(0, 1, 3, 2, 4, 5)
    return np.ascontiguousarray(out)


LAST_RESULT = None


def kernel(inp, filters):
    global LAST_RESULT
    inp = np.asarray(inp, dtype=np.float32)
    filters = np.asarray(filters, dtype=np.float32)
    xins, w = _host_prep(inp, filters)
    nc = _build_program()
    if MM_DTYPE == "bfloat16":
        import ml_dtypes
        xins = [x.astype(ml_dtypes.bfloat16) for x in xins]
        w = w.astype(ml_dtypes.bfloat16)
    in_maps = [{"xin": xins[c], "win": w} for c in range(NCORES)]

    if os.environ.get("BASS_KERNEL_SIM"):
        from concourse.bass_interp import CoreSim
        results = []
        for c in range(NCORES):
            sim = CoreSim(nc)
            for k, v in in_maps[c].items():
                sim.tensor(k)[:] = v
            sim.simulate()
            results.append({"out": sim.tensor("out").copy()})
    else:
        try:
            from concourse._compat import axon_active
            use_fast = axon_active()
        except Exception:
            use_fast = True
        if use_fast:
            results = _run_fast(nc, in_maps)
        else:
            from concourse.bass_utils import run_bass_kernel_spmd
            res = run_bass_kernel_spmd(nc, in_maps, list(range(NCORES)))
            LAST_RESULT = res
            results = res.results
    return _assemble(results)


_RUNNER = {}


def _get_runner(nc):
    """Build (once) a jitted 8-core shard_map executable for `nc`."""
    if id(nc) in _RUNNER:
        return _RUNNER[id(nc)]
    import jax
    import numpy as _np
    from jax.sharding import Mesh, PartitionSpec
    from jax.experimental.shard_map import shard_map
    from concourse import mybir
    from concourse.bass2jax import (_bass_exec_p, install_neuronx_cc_hook,
                                    partition_id_tensor)

    install_neuronx_cc_hook()
    partition_name = (nc.partition_id_tensor.name
                      if nc.partition_id_tensor else None)
    in_names, out_names, out_avals, zero_shapes = [], [], [], []
    for alloc in nc.m.functions[0].allocations:
        if not isinstance(alloc, mybir.MemoryLocationSet):
            continue
        name = alloc.memorylocations[0].name
        if alloc.kind == "ExternalInput":
            if name != partition_name:
                in_names.append(name)
        elif alloc.kind == "ExternalOutput":
            shape = tuple(alloc.tensor_shape)
            dtype = mybir.dt.np(alloc.dtype)
            out_names.append(name)
            out_avals.append(jax.core.ShapedArray(shape, dtype))
            zero_shapes.append((shape, dtype))
    n_params = len(in_names)
    all_in_names = list(in_names) + list(out_names)
    if partition_name is not None:
        all_in_names.append(partition_name)
    donate = tuple(range(n_params, n_params + len(out_names)))

    def _body(*args):
        operands = list(args)
        if partition_name is not None:
            operands.append(partition_id_tensor())
        outs = _bass_exec_p.bind(
            *operands,
            out_avals=tuple(out_avals),
            in_names=tuple(all_in_names),
            out_names=tuple(out_names),
            lowering_input_output_aliases=(),
            sim_require_finite=True,
            sim_require_nnan=True,
            nc=nc,
        )
        return tuple(outs)

    devices = jax.devices()[:NCORES]
    mesh = Mesh(_np.asarray(devices), ("core",))
    nio = n_params + len(out_names)
    sharded = jax.jit(
        shard_map(_body, mesh=mesh, in_specs=(PartitionSpec("core"),) * nio,
                  out_specs=(PartitionSpec("core"),) * len(out_names),
                  check_rep=False),
        donate_argnums=donate, keep_unused=True)
    r = (sharded, in_names, out_names, zero_shapes)
    _RUNNER[id(nc)] = r
    return r


def _run_fast(nc, in_maps):
    import numpy as _np
    sharded, in_names, out_names, zero_shapes = _get_runner(nc)
    concat_in = [
        _np.concatenate([_np.asarray(in_maps[c][nm]) for c in range(NCORES)],
                        axis=0)
        for nm in in_names
    ]
    concat_zeros = [_np.zeros((NCORES * s[0], *s[1:]), d)
                    for s, d in zero_shapes]
    out_arrs = sharded(*concat_in, *concat_zeros)
    per_core_shapes = [s for s, _ in zero_shapes]
    return [
        {nm: _np.asarray(out_arrs[i]).reshape(NCORES, *per_core_shapes[i])[c]
         for i, nm in enumerate(out_names)}
        for c in range(NCORES)
    ]
